# revision 16
# baseline (speedup 1.0000x reference)
"""Trainium2 Bass kernel for EntityConsolidationHead (pair MLP + BCE loss).

reference math (per pair p):
    a = emb[pair_a[p]]; b = emb[pair_b[p]]              # [768]
    feats = [a, b, |a-b|, a*b]                          # [3072]
    h = gelu_exact(feats @ W1 + b1)                     # [768]
    logit = h @ W2 + b2
    loss = mean(relu(logit) - logit*y + softplus(-|logit|))

Sharding: data-parallel over pairs across 8 NeuronCores; node_embeddings
and MLP weights replicated. Per-core partial loss sums are reduced on host.
"""

import sys

sys.path.insert(0, "/opt/trn_rl_repo")

import numpy as np

N_CORES = 8
H = 768
H4 = 4 * H  # 3072
P = 128  # SBUF partitions / pair-tile size
KC = H4 // P  # 24 contraction chunks of 128
N_NODES = 50000
N_PAIRS = 100000

_CACHE = {}


def _build_program(n_nodes: int, n_tiles: int):
    """Build the SPMD Bass program for one core handling n_tiles*128 pairs."""
    import concourse.bass as bass
    import concourse.mybir as mybir
    import concourse.tile as tile
    from concourse.masks import make_identity
    from concourse.vector_clock import ScopedClock

    TC = tile.TileContext
    _ = ScopedClock  # retained import

    f32 = mybir.dt.float32
    bf16 = mybir.dt.bfloat16
    i32 = mybir.dt.int32
    NP = n_tiles * P

    nc = bass.Bass()
    emb = nc.declare_dram_parameter("emb", [n_nodes, H], f32, isOutput=False)
    pa = nc.declare_dram_parameter("pa", [NP], i32, isOutput=False)
    pb = nc.declare_dram_parameter("pb", [NP], i32, isOutput=False)
    w1 = nc.declare_dram_parameter("w1", [H4, H], f32, isOutput=False)
    b1 = nc.declare_dram_parameter("b1", [H], f32, isOutput=False)
    w2 = nc.declare_dram_parameter("w2", [H], f32, isOutput=False)
    b2 = nc.declare_dram_parameter("b2", [1], f32, isOutput=False)
    logits_out = nc.declare_dram_parameter("logits_out", [NP], f32, isOutput=True)

    AF = mybir.ActivationFunctionType
    OP = mybir.AluOpType

    with TC(nc) as tc:
        with (
            tc.tile_pool(name="const", bufs=1) as cpool,
            tc.tile_pool(name="work", bufs=1) as wpool,
            tc.tile_pool(name="io", bufs=4) as iopool,
            tc.tile_pool(name="mid", bufs=3) as midpool,
            tc.tile_pool(name="psum_t", bufs=2, space="PSUM") as tpsum,
            tc.tile_pool(name="psum_h", bufs=4, space="PSUM") as hpsum,
        ):
            # ---- constants / weights resident in SBUF ----
            ident = cpool.tile([P, P], f32, tag="ident")
            make_identity(nc, ident[:])
            identb = cpool.tile([P, P], bf16, tag="identb")
            make_identity(nc, identb[:])
            w1_sb = cpool.tile([P, KC * H], bf16, tag="w1sb")  # chunk k at [:, k*H:(k+1)*H]
            nc.gpsimd.dma_start(
                out=w1_sb[:], in_=w1.rearrange("(k p) n -> p k n", p=P)
            )
            b1_rep = cpool.tile([P, H], f32, tag="b1rep")
            nc.sync.dma_start(out=b1_rep[:], in_=b1[None, :].to_broadcast([P, H]))
            w2_rep = cpool.tile([P, H], f32, tag="w2rep")
            nc.sync.dma_start(out=w2_rep[:], in_=w2[None, :].to_broadcast([P, H]))
            b2_rep = cpool.tile([P, 1], f32, tag="b2rep")
            nc.sync.dma_start(out=b2_rep[:], in_=b2[None, :].to_broadcast([P, 1]))
            logit_sb = wpool.tile([P, n_tiles], f32, tag="logit")
            ia_all = cpool.tile([P, n_tiles], i32, tag="iaall")
            nc.sync.dma_start(out=ia_all[:], in_=pa.rearrange("(t l) -> l t", l=P))
            ib_all = cpool.tile([P, n_tiles], i32, tag="iball")
            nc.sync.dma_start(out=ib_all[:], in_=pb.rearrange("(t l) -> l t", l=P))

            # ---- main loop over pair tiles ----
            for t in range(n_tiles):
                ga = iopool.tile([P, H], bf16, tag="ga")
                gb = iopool.tile([P, H], bf16, tag="gb")
                nc.gpsimd.indirect_dma_start(
                    out=ga[:], out_offset=None, in_=emb[:],
                    in_offset=bass.IndirectOffsetOnAxis(ap=ia_all[:, t : t + 1], axis=0),
                )
                nc.gpsimd.indirect_dma_start(
                    out=gb[:], out_offset=None, in_=emb[:],
                    in_offset=bass.IndirectOffsetOnAxis(ap=ib_all[:, t : t + 1], axis=0),
                )

                # PE-transpose (bf16, 1 cyc/row) to feature-major; a chunks 0-5, b chunks 6-11
                abT = midpool.tile([P, 2 * H], bf16, tag="abT")
                tp = tpsum.tile([P, 2 * H], bf16, tag="tp", space="PSUM")
                for ci, src in ((0, ga), (6, gb)):
                    for c in range(6):
                        nc.tensor.transpose(
                            out=tp[:, (ci + c) * P : (ci + c + 1) * P],
                            in_=src[:, c * P : (c + 1) * P],
                            identity=identb[:],
                        )
                nc.scalar.copy(out=abT[:], in_=tp[:])
                aT = abT[:, :H]
                bT = abT[:, H:]

                dif = midpool.tile([P, H], bf16, tag="dif")
                prd = midpool.tile([P, H], bf16, tag="prd")
                nc.vector.tensor_sub(out=dif[:], in0=aT, in1=bT)
                nc.scalar.activation(out=dif[:], in_=dif[:], func=AF.Abs)
                nc.vector.tensor_mul(out=prd[:], in0=aT, in1=bT)

                # h = gelu(F @ W1 + b1), pairs-major [128, 768]
                h_sb = midpool.tile([P, H], f32, tag="h")
                for half in range(2):
                    ph = hpsum.tile([P, 384], f32, tag="ph", space="PSUM")
                    nsl = slice(half * 384, (half + 1) * 384)
                    for k in range(KC):
                        if k < 12:
                            lhs = abT[:, k * P : (k + 1) * P]
                        elif k < 18:
                            lhs = dif[:, (k - 12) * P : (k - 11) * P]
                        else:
                            lhs = prd[:, (k - 18) * P : (k - 17) * P]
                        nc.tensor.matmul(
                            out=ph[:],
                            lhsT=lhs,
                            rhs=w1_sb[:, k * H + half * 384 : k * H + half * 384 + 384],
                            start=(k == 0),
                            stop=(k == KC - 1),
                        )
                    nc.vector.tensor_add(
                        out=h_sb[:, nsl], in0=ph[:], in1=b1_rep[:, nsl]
                    )
                    nc.scalar.activation(
                        out=h_sb[:, nsl], in_=h_sb[:, nsl], func=AF.Gelu
                    )

                # logit[p] = sum_f h[p,f] * w2[f]  (fused mul + free-dim sum)
                trash = midpool.tile([P, H], f32, tag="trash")
                nc.vector.scalar_tensor_tensor(
                    out=trash[:],
                    in0=h_sb[:],
                    scalar=0.0,
                    in1=w2_rep[:],
                    op0=OP.add,
                    op1=OP.mult,
                    accum_out=logit_sb[:, t : t + 1],
                )

            # ---- epilogue: +b2, logits out ----
            nc.vector.tensor_scalar(
                out=logit_sb[:], in0=logit_sb[:], scalar1=b2_rep[:, :1],
                scalar2=None, op0=OP.add,
            )
            # logits out: transpose [128, T] -> [T, 128] then contiguous DMA
            ltp = tpsum.tile([P, P], f32, tag="tp", space="PSUM")
            nc.tensor.transpose(out=ltp[:n_tiles, :], in_=logit_sb[:], identity=ident[:])
            lo_sb = wpool.tile([P, P], f32, tag="losb")
            nc.scalar.copy(out=lo_sb[:n_tiles, :], in_=ltp[:n_tiles, :])
            nc.sync.dma_start(
                out=logits_out.rearrange("(t l) -> t l", l=P), in_=lo_sb[:n_tiles, :]
            )

    _split_multi_waits(nc, mybir)
    return nc


def _split_multi_waits(nc, mybir):
    """This walrus build allows at most one sync-wait per instruction:
    hoist extra waits onto same-engine NOPs placed just before."""
    uid = 0
    for bb in nc.main_func.blocks:
        new_list = []
        for inst in bb.instructions:
            si = inst.sync_info
            if si is not None and si.on_wait and len(si.on_wait) > 1:
                waits = list(si.on_wait)
                for sw in waits[:-1]:
                    uid += 1
                    new_list.append(
                        mybir.InstNoOp(
                            name=f"I-wsplit-{uid}",
                            sync_info=mybir.SyncInfo(on_wait=[sw], on_update=[]),
                            bass_nofuse=True,
                            engine=inst.engine,
                        )
                    )
                si.on_wait = waits[-1:]
            new_list.append(inst)
        bb.instructions[:] = new_list


def kernel(node_embeddings, pair_a, pair_b, labels, W1, b1, W2, b2):
    from concourse.bass_utils import run_bass_kernel_spmd

    node_embeddings = np.ascontiguousarray(np.asarray(node_embeddings, dtype=np.float32))
    pair_a = np.asarray(pair_a).astype(np.int32)
    pair_b = np.asarray(pair_b).astype(np.int32)
    labels_in = np.asarray(labels)
    W1 = np.ascontiguousarray(np.asarray(W1, dtype=np.float32))
    b1v = np.asarray(b1, dtype=np.float32).reshape(-1)
    W2v = np.asarray(W2, dtype=np.float32).reshape(-1)
    b2v = np.asarray(b2, dtype=np.float32).reshape(-1)

    n_nodes = node_embeddings.shape[0]
    n_pairs = pair_a.shape[0]
    per_core = -(-n_pairs // N_CORES)
    n_tiles = -(-per_core // P)
    npc = n_tiles * P  # padded pairs per core

    key = (n_nodes, n_tiles)
    if key not in _CACHE:
        _CACHE[key] = _build_program(n_nodes, n_tiles)
    nc = _CACHE[key]

    lab_f = labels_in.astype(np.float32)
    in_maps = []
    for c in range(N_CORES):
        lo = c * per_core
        hi = min(lo + per_core, n_pairs)
        n = hi - lo
        pa_c = np.zeros(npc, dtype=np.int32)
        pb_c = np.zeros(npc, dtype=np.int32)
        pa_c[:n] = pair_a[lo:hi]
        pb_c[:n] = pair_b[lo:hi]
        in_maps.append(
            {
                "emb": node_embeddings,
                "pa": pa_c,
                "pb": pb_c,
                "w1": W1,
                "b1": b1v,
                "w2": W2v,
                "b2": b2v,
            }
        )

    res = run_bass_kernel_spmd(nc, in_maps, list(range(N_CORES)))
    global LAST_RESULTS
    LAST_RESULTS = res

    logits = np.empty(n_pairs, dtype=np.float32)
    for c in range(N_CORES):
        lo = c * per_core
        hi = min(lo + per_core, n_pairs)
        logits[lo:hi] = res.results[c]["logits_out"][: hi - lo]

    # BCE-with-logits, mean reduction (stable form), as part of unsharding
    l64 = logits.astype(np.float64)
    y64 = lab_f.astype(np.float64)
    terms = np.maximum(l64, 0.0) - l64 * y64 + np.log1p(np.exp(-np.abs(l64)))
    loss = np.float32(terms.mean())
    return loss, logits


# revision 18
# speedup vs baseline: 1.0397x; 1.0397x over previous
"""Trainium2 Bass kernel for EntityConsolidationHead (pair MLP + BCE loss).

reference math (per pair p):
    a = emb[pair_a[p]]; b = emb[pair_b[p]]              # [768]
    feats = [a, b, |a-b|, a*b]                          # [3072]
    h = gelu_exact(feats @ W1 + b1)                     # [768]
    logit = h @ W2 + b2
    loss = mean(relu(logit) - logit*y + softplus(-|logit|))

Sharding: data-parallel over pairs across 8 NeuronCores; node_embeddings
and MLP weights replicated. Per-core partial loss sums are reduced on host.
"""

import sys

sys.path.insert(0, "/opt/trn_rl_repo")

import numpy as np

N_CORES = 8
H = 768
H4 = 4 * H  # 3072
P = 128  # SBUF partitions / pair-tile size
KC = H4 // P  # 24 contraction chunks of 128
N_NODES = 50000
N_PAIRS = 100000

_CACHE = {}


def _build_program(n_nodes: int, n_tiles: int):
    """Build the SPMD Bass program for one core handling n_tiles*128 pairs."""
    import concourse.bass as bass
    import concourse.mybir as mybir
    import concourse.tile as tile
    from concourse.masks import make_identity
    from concourse.vector_clock import ScopedClock

    TC = tile.TileContext
    _ = ScopedClock  # retained import

    f32 = mybir.dt.float32
    bf16 = mybir.dt.bfloat16
    i32 = mybir.dt.int32
    NP = n_tiles * P

    nc = bass.Bass()
    emb = nc.declare_dram_parameter("emb", [n_nodes, H], f32, isOutput=False)
    pa = nc.declare_dram_parameter("pa", [NP], i32, isOutput=False)
    pb = nc.declare_dram_parameter("pb", [NP], i32, isOutput=False)
    w1 = nc.declare_dram_parameter("w1", [H4, H], f32, isOutput=False)
    b1 = nc.declare_dram_parameter("b1", [H], f32, isOutput=False)
    w2 = nc.declare_dram_parameter("w2", [H], f32, isOutput=False)
    b2 = nc.declare_dram_parameter("b2", [1], f32, isOutput=False)
    logits_out = nc.declare_dram_parameter("logits_out", [NP], f32, isOutput=True)

    AF = mybir.ActivationFunctionType
    OP = mybir.AluOpType

    with TC(nc) as tc:
        with (
            tc.tile_pool(name="const", bufs=1) as cpool,
            tc.tile_pool(name="work", bufs=1) as wpool,
            tc.tile_pool(name="io", bufs=4) as iopool,
            tc.tile_pool(name="mid", bufs=3) as midpool,
            tc.tile_pool(name="psum_t", bufs=2, space="PSUM") as tpsum,
            tc.tile_pool(name="psum_h", bufs=4, space="PSUM") as hpsum,
        ):
            # ---- constants / weights resident in SBUF ----
            ident = cpool.tile([P, P], f32, tag="ident")
            make_identity(nc, ident[:])
            identb = cpool.tile([P, P], bf16, tag="identb")
            make_identity(nc, identb[:])
            w1_sb = cpool.tile([P, KC * H], bf16, tag="w1sb")  # chunk k at [:, k*H:(k+1)*H]
            nc.gpsimd.dma_start(
                out=w1_sb[:], in_=w1.rearrange("(k p) n -> p k n", p=P)
            )
            b1_rep = cpool.tile([P, H], f32, tag="b1rep")
            nc.sync.dma_start(out=b1_rep[:], in_=b1[None, :].to_broadcast([P, H]))
            w2_rep = cpool.tile([P, H], f32, tag="w2rep")
            nc.sync.dma_start(out=w2_rep[:], in_=w2[None, :].to_broadcast([P, H]))
            b2_rep = cpool.tile([P, 1], f32, tag="b2rep")
            nc.sync.dma_start(out=b2_rep[:], in_=b2[None, :].to_broadcast([P, 1]))
            logit_sb = wpool.tile([P, n_tiles], f32, tag="logit")
            ia_all = cpool.tile([P, n_tiles], i32, tag="iaall")
            nc.sync.dma_start(out=ia_all[:], in_=pa.rearrange("(t l) -> l t", l=P))
            ib_all = cpool.tile([P, n_tiles], i32, tag="iball")
            nc.sync.dma_start(out=ib_all[:], in_=pb.rearrange("(t l) -> l t", l=P))

            # ---- main loop over pair tiles ----
            for t in range(n_tiles):
                ga = iopool.tile([P, H], bf16, tag="ga")
                gb = iopool.tile([P, H], bf16, tag="gb")
                nc.gpsimd.indirect_dma_start(
                    out=ga[:], out_offset=None, in_=emb[:],
                    in_offset=bass.IndirectOffsetOnAxis(ap=ia_all[:, t : t + 1], axis=0),
                )
                nc.gpsimd.indirect_dma_start(
                    out=gb[:], out_offset=None, in_=emb[:],
                    in_offset=bass.IndirectOffsetOnAxis(ap=ib_all[:, t : t + 1], axis=0),
                )

                # PE-transpose (bf16, 1 cyc/row) to feature-major [768, 128]
                aT = midpool.tile([P, H], bf16, tag="aT")
                bT = midpool.tile([P, H], bf16, tag="bT")
                for src, dstT, tg in ((ga, aT, "tpa"), (gb, bT, "tpb")):
                    for half in range(2):
                        tp = tpsum.tile([P, 3 * P], bf16, tag=tg, space="PSUM")
                        for c in range(3):
                            cc = half * 3 + c
                            nc.tensor.transpose(
                                out=tp[:, c * P : (c + 1) * P],
                                in_=src[:, cc * P : (cc + 1) * P],
                                identity=identb[:],
                            )
                        nc.scalar.copy(
                            out=dstT[:, half * 3 * P : (half + 1) * 3 * P], in_=tp[:]
                        )

                dif = midpool.tile([P, H], bf16, tag="dif")
                prd = midpool.tile([P, H], bf16, tag="prd")
                nc.vector.tensor_sub(out=dif[:], in0=aT[:], in1=bT[:])
                nc.scalar.activation(out=dif[:], in_=dif[:], func=AF.Abs)
                nc.vector.tensor_mul(out=prd[:], in0=aT[:], in1=bT[:])

                # h = gelu(F @ W1 + b1), pairs-major [128, 768]
                h_sb = midpool.tile([P, H], f32, tag="h")
                for half in range(2):
                    ph = hpsum.tile([P, 384], f32, tag="ph", space="PSUM")
                    nsl = slice(half * 384, (half + 1) * 384)
                    for k in range(KC):
                        if k < 6:
                            lhs = aT[:, k * P : (k + 1) * P]
                        elif k < 12:
                            lhs = bT[:, (k - 6) * P : (k - 5) * P]
                        elif k < 18:
                            lhs = dif[:, (k - 12) * P : (k - 11) * P]
                        else:
                            lhs = prd[:, (k - 18) * P : (k - 17) * P]
                        nc.tensor.matmul(
                            out=ph[:],
                            lhsT=lhs,
                            rhs=w1_sb[:, k * H + half * 384 : k * H + half * 384 + 384],
                            start=(k == 0),
                            stop=(k == KC - 1),
                        )
                    nc.vector.tensor_add(
                        out=h_sb[:, nsl], in0=ph[:], in1=b1_rep[:, nsl]
                    )
                    nc.scalar.activation(
                        out=h_sb[:, nsl], in_=h_sb[:, nsl], func=AF.Gelu
                    )

                # logit[p] = sum_f h[p,f] * w2[f]  (fused mul + free-dim sum)
                trash = midpool.tile([P, H], f32, tag="trash")
                nc.vector.scalar_tensor_tensor(
                    out=trash[:],
                    in0=h_sb[:],
                    scalar=0.0,
                    in1=w2_rep[:],
                    op0=OP.add,
                    op1=OP.mult,
                    accum_out=logit_sb[:, t : t + 1],
                )

            # ---- epilogue: +b2, logits out ----
            nc.vector.tensor_scalar(
                out=logit_sb[:], in0=logit_sb[:], scalar1=b2_rep[:, :1],
                scalar2=None, op0=OP.add,
            )
            # logits out: transpose [128, T] -> [T, 128] then contiguous DMA
            ltp = tpsum.tile([P, P], f32, tag="tpa", space="PSUM")
            nc.tensor.transpose(out=ltp[:n_tiles, :], in_=logit_sb[:], identity=ident[:])
            lo_sb = wpool.tile([P, P], f32, tag="losb")
            nc.scalar.copy(out=lo_sb[:n_tiles, :], in_=ltp[:n_tiles, :])
            nc.sync.dma_start(
                out=logits_out.rearrange("(t l) -> t l", l=P), in_=lo_sb[:n_tiles, :]
            )

    _split_multi_waits(nc, mybir)
    return nc


def _split_multi_waits(nc, mybir):
    """This walrus build allows at most one sync-wait per instruction:
    hoist extra waits onto same-engine NOPs placed just before."""
    uid = 0
    for bb in nc.main_func.blocks:
        new_list = []
        for inst in bb.instructions:
            si = inst.sync_info
            if si is not None and si.on_wait and len(si.on_wait) > 1:
                waits = list(si.on_wait)
                for sw in waits[:-1]:
                    uid += 1
                    new_list.append(
                        mybir.InstNoOp(
                            name=f"I-wsplit-{uid}",
                            sync_info=mybir.SyncInfo(on_wait=[sw], on_update=[]),
                            bass_nofuse=True,
                            engine=inst.engine,
                        )
                    )
                si.on_wait = waits[-1:]
            new_list.append(inst)
        bb.instructions[:] = new_list


def kernel(node_embeddings, pair_a, pair_b, labels, W1, b1, W2, b2):
    from concourse.bass_utils import run_bass_kernel_spmd

    node_embeddings = np.ascontiguousarray(np.asarray(node_embeddings, dtype=np.float32))
    pair_a = np.asarray(pair_a).astype(np.int32)
    pair_b = np.asarray(pair_b).astype(np.int32)
    labels_in = np.asarray(labels)
    W1 = np.ascontiguousarray(np.asarray(W1, dtype=np.float32))
    b1v = np.asarray(b1, dtype=np.float32).reshape(-1)
    W2v = np.asarray(W2, dtype=np.float32).reshape(-1)
    b2v = np.asarray(b2, dtype=np.float32).reshape(-1)

    n_nodes = node_embeddings.shape[0]
    n_pairs = pair_a.shape[0]
    per_core = -(-n_pairs // N_CORES)
    n_tiles = -(-per_core // P)
    npc = n_tiles * P  # padded pairs per core

    key = (n_nodes, n_tiles)
    if key not in _CACHE:
        _CACHE[key] = _build_program(n_nodes, n_tiles)
    nc = _CACHE[key]

    lab_f = labels_in.astype(np.float32)
    in_maps = []
    for c in range(N_CORES):
        lo = c * per_core
        hi = min(lo + per_core, n_pairs)
        n = hi - lo
        pa_c = np.zeros(npc, dtype=np.int32)
        pb_c = np.zeros(npc, dtype=np.int32)
        pa_c[:n] = pair_a[lo:hi]
        pb_c[:n] = pair_b[lo:hi]
        in_maps.append(
            {
                "emb": node_embeddings,
                "pa": pa_c,
                "pb": pb_c,
                "w1": W1,
                "b1": b1v,
                "w2": W2v,
                "b2": b2v,
            }
        )

    res = run_bass_kernel_spmd(nc, in_maps, list(range(N_CORES)))
    global LAST_RESULTS
    LAST_RESULTS = res

    logits = np.empty(n_pairs, dtype=np.float32)
    for c in range(N_CORES):
        lo = c * per_core
        hi = min(lo + per_core, n_pairs)
        logits[lo:hi] = res.results[c]["logits_out"][: hi - lo]

    # BCE-with-logits, mean reduction (stable form), as part of unsharding
    l64 = logits.astype(np.float64)
    y64 = lab_f.astype(np.float64)
    terms = np.maximum(l64, 0.0) - l64 * y64 + np.log1p(np.exp(-np.abs(l64)))
    loss = np.float32(terms.mean())
    return loss, logits


# revision 19
# speedup vs baseline: 1.0457x; 1.0058x over previous
"""Trainium2 Bass kernel for EntityConsolidationHead (pair MLP + BCE loss).

reference math (per pair p):
    a = emb[pair_a[p]]; b = emb[pair_b[p]]              # [768]
    feats = [a, b, |a-b|, a*b]                          # [3072]
    h = gelu_exact(feats @ W1 + b1)                     # [768]
    logit = h @ W2 + b2
    loss = mean(relu(logit) - logit*y + softplus(-|logit|))

Sharding: data-parallel over pairs across 8 NeuronCores; node_embeddings
and MLP weights replicated. Per-core partial loss sums are reduced on host.
"""

import sys

sys.path.insert(0, "/opt/trn_rl_repo")

import numpy as np

N_CORES = 8
H = 768
H4 = 4 * H  # 3072
P = 128  # SBUF partitions / pair-tile size
KC = H4 // P  # 24 contraction chunks of 128
N_NODES = 50000
N_PAIRS = 100000

_CACHE = {}


def _build_program(n_nodes: int, n_tiles: int):
    """Build the SPMD Bass program for one core handling n_tiles*128 pairs."""
    import concourse.bass as bass
    import concourse.mybir as mybir
    import concourse.tile as tile
    from concourse.masks import make_identity
    from concourse.vector_clock import ScopedClock

    TC = tile.TileContext
    _ = ScopedClock  # retained import

    f32 = mybir.dt.float32
    bf16 = mybir.dt.bfloat16
    i32 = mybir.dt.int32
    NP = n_tiles * P

    nc = bass.Bass()
    emb = nc.declare_dram_parameter("emb", [n_nodes, H], f32, isOutput=False)
    pa = nc.declare_dram_parameter("pa", [NP], i32, isOutput=False)
    pb = nc.declare_dram_parameter("pb", [NP], i32, isOutput=False)
    w1 = nc.declare_dram_parameter("w1", [H4, H], f32, isOutput=False)
    b1 = nc.declare_dram_parameter("b1", [H], f32, isOutput=False)
    w2 = nc.declare_dram_parameter("w2", [H], f32, isOutput=False)
    b2 = nc.declare_dram_parameter("b2", [1], f32, isOutput=False)
    logits_out = nc.declare_dram_parameter("logits_out", [NP], f32, isOutput=True)

    AF = mybir.ActivationFunctionType
    OP = mybir.AluOpType

    with TC(nc) as tc:
        with (
            tc.tile_pool(name="const", bufs=1) as cpool,
            tc.tile_pool(name="work", bufs=1) as wpool,
            tc.tile_pool(name="io", bufs=4) as iopool,
            tc.tile_pool(name="mid", bufs=3) as midpool,
            tc.tile_pool(name="psum_t", bufs=2, space="PSUM") as tpsum,
            tc.tile_pool(name="psum_h", bufs=4, space="PSUM") as hpsum,
        ):
            # ---- constants / weights resident in SBUF ----
            ident = cpool.tile([P, P], f32, tag="ident")
            make_identity(nc, ident[:])
            identb = cpool.tile([P, P], bf16, tag="identb")
            make_identity(nc, identb[:])
            w1_sb = cpool.tile([P, KC * H], bf16, tag="w1sb")  # chunk k at [:, k*H:(k+1)*H]
            w1_view = w1.rearrange("(k p) n -> k p n", p=P)
            for k in range(KC):
                nc.gpsimd.dma_start(
                    out=w1_sb[:, k * H : (k + 1) * H], in_=w1_view[k]
                )
            b1_rep = cpool.tile([P, H], f32, tag="b1rep")
            nc.sync.dma_start(out=b1_rep[:], in_=b1[None, :].to_broadcast([P, H]))
            w2_rep = cpool.tile([P, H], f32, tag="w2rep")
            nc.sync.dma_start(out=w2_rep[:], in_=w2[None, :].to_broadcast([P, H]))
            b2_rep = cpool.tile([P, 1], f32, tag="b2rep")
            nc.sync.dma_start(out=b2_rep[:], in_=b2[None, :].to_broadcast([P, 1]))
            logit_sb = wpool.tile([P, n_tiles], f32, tag="logit")
            ia_all = cpool.tile([P, n_tiles], i32, tag="iaall")
            nc.sync.dma_start(out=ia_all[:], in_=pa.rearrange("(t l) -> l t", l=P))
            ib_all = cpool.tile([P, n_tiles], i32, tag="iball")
            nc.sync.dma_start(out=ib_all[:], in_=pb.rearrange("(t l) -> l t", l=P))

            # ---- main loop over pair tiles ----
            for t in range(n_tiles):
                ga = iopool.tile([P, H], bf16, tag="ga")
                gb = iopool.tile([P, H], bf16, tag="gb")
                nc.gpsimd.indirect_dma_start(
                    out=ga[:], out_offset=None, in_=emb[:],
                    in_offset=bass.IndirectOffsetOnAxis(ap=ia_all[:, t : t + 1], axis=0),
                )
                nc.gpsimd.indirect_dma_start(
                    out=gb[:], out_offset=None, in_=emb[:],
                    in_offset=bass.IndirectOffsetOnAxis(ap=ib_all[:, t : t + 1], axis=0),
                )

                # PE-transpose (bf16, 1 cyc/row) to feature-major [768, 128]
                aT = midpool.tile([P, H], bf16, tag="aT")
                bT = midpool.tile([P, H], bf16, tag="bT")
                for src, dstT, tg in ((ga, aT, "tpa"), (gb, bT, "tpb")):
                    for half in range(2):
                        tp = tpsum.tile([P, 3 * P], bf16, tag=tg, space="PSUM")
                        for c in range(3):
                            cc = half * 3 + c
                            nc.tensor.transpose(
                                out=tp[:, c * P : (c + 1) * P],
                                in_=src[:, cc * P : (cc + 1) * P],
                                identity=identb[:],
                            )
                        nc.scalar.copy(
                            out=dstT[:, half * 3 * P : (half + 1) * 3 * P], in_=tp[:]
                        )

                dif = midpool.tile([P, H], bf16, tag="dif")
                prd = midpool.tile([P, H], bf16, tag="prd")
                nc.vector.tensor_sub(out=dif[:], in0=aT[:], in1=bT[:])
                nc.scalar.activation(out=dif[:], in_=dif[:], func=AF.Abs)
                nc.vector.tensor_mul(out=prd[:], in0=aT[:], in1=bT[:])

                # h = gelu(F @ W1 + b1), pairs-major [128, 768]
                h_sb = midpool.tile([P, H], f32, tag="h")
                for half in range(2):
                    ph = hpsum.tile([P, 384], f32, tag="ph", space="PSUM")
                    nsl = slice(half * 384, (half + 1) * 384)
                    for k in range(KC):
                        if k < 6:
                            lhs = aT[:, k * P : (k + 1) * P]
                        elif k < 12:
                            lhs = bT[:, (k - 6) * P : (k - 5) * P]
                        elif k < 18:
                            lhs = dif[:, (k - 12) * P : (k - 11) * P]
                        else:
                            lhs = prd[:, (k - 18) * P : (k - 17) * P]
                        nc.tensor.matmul(
                            out=ph[:],
                            lhsT=lhs,
                            rhs=w1_sb[:, k * H + half * 384 : k * H + half * 384 + 384],
                            start=(k == 0),
                            stop=(k == KC - 1),
                        )
                    nc.vector.tensor_add(
                        out=h_sb[:, nsl], in0=ph[:], in1=b1_rep[:, nsl]
                    )
                    nc.scalar.activation(
                        out=h_sb[:, nsl], in_=h_sb[:, nsl], func=AF.Gelu
                    )

                # logit[p] = sum_f h[p,f] * w2[f]  (fused mul + free-dim sum)
                trash = midpool.tile([P, H], f32, tag="trash")
                nc.vector.scalar_tensor_tensor(
                    out=trash[:],
                    in0=h_sb[:],
                    scalar=0.0,
                    in1=w2_rep[:],
                    op0=OP.add,
                    op1=OP.mult,
                    accum_out=logit_sb[:, t : t + 1],
                )

            # ---- epilogue: +b2, logits out ----
            nc.vector.tensor_scalar(
                out=logit_sb[:], in0=logit_sb[:], scalar1=b2_rep[:, :1],
                scalar2=None, op0=OP.add,
            )
            # logits out: transpose [128, T] -> [T, 128] then contiguous DMA
            ltp = tpsum.tile([P, P], f32, tag="tpa", space="PSUM")
            nc.tensor.transpose(out=ltp[:n_tiles, :], in_=logit_sb[:], identity=ident[:])
            lo_sb = wpool.tile([P, P], f32, tag="losb")
            nc.scalar.copy(out=lo_sb[:n_tiles, :], in_=ltp[:n_tiles, :])
            nc.sync.dma_start(
                out=logits_out.rearrange("(t l) -> t l", l=P), in_=lo_sb[:n_tiles, :]
            )

    _split_multi_waits(nc, mybir)
    return nc


def _split_multi_waits(nc, mybir):
    """This walrus build allows at most one sync-wait per instruction:
    hoist extra waits onto same-engine NOPs placed just before."""
    uid = 0
    for bb in nc.main_func.blocks:
        new_list = []
        for inst in bb.instructions:
            si = inst.sync_info
            if si is not None and si.on_wait and len(si.on_wait) > 1:
                waits = list(si.on_wait)
                for sw in waits[:-1]:
                    uid += 1
                    new_list.append(
                        mybir.InstNoOp(
                            name=f"I-wsplit-{uid}",
                            sync_info=mybir.SyncInfo(on_wait=[sw], on_update=[]),
                            bass_nofuse=True,
                            engine=inst.engine,
                        )
                    )
                si.on_wait = waits[-1:]
            new_list.append(inst)
        bb.instructions[:] = new_list


def kernel(node_embeddings, pair_a, pair_b, labels, W1, b1, W2, b2):
    from concourse.bass_utils import run_bass_kernel_spmd

    node_embeddings = np.ascontiguousarray(np.asarray(node_embeddings, dtype=np.float32))
    pair_a = np.asarray(pair_a).astype(np.int32)
    pair_b = np.asarray(pair_b).astype(np.int32)
    labels_in = np.asarray(labels)
    W1 = np.ascontiguousarray(np.asarray(W1, dtype=np.float32))
    b1v = np.asarray(b1, dtype=np.float32).reshape(-1)
    W2v = np.asarray(W2, dtype=np.float32).reshape(-1)
    b2v = np.asarray(b2, dtype=np.float32).reshape(-1)

    n_nodes = node_embeddings.shape[0]
    n_pairs = pair_a.shape[0]
    per_core = -(-n_pairs // N_CORES)
    n_tiles = -(-per_core // P)
    npc = n_tiles * P  # padded pairs per core

    key = (n_nodes, n_tiles)
    if key not in _CACHE:
        _CACHE[key] = _build_program(n_nodes, n_tiles)
    nc = _CACHE[key]

    lab_f = labels_in.astype(np.float32)
    in_maps = []
    for c in range(N_CORES):
        lo = c * per_core
        hi = min(lo + per_core, n_pairs)
        n = hi - lo
        pa_c = np.zeros(npc, dtype=np.int32)
        pb_c = np.zeros(npc, dtype=np.int32)
        pa_c[:n] = pair_a[lo:hi]
        pb_c[:n] = pair_b[lo:hi]
        in_maps.append(
            {
                "emb": node_embeddings,
                "pa": pa_c,
                "pb": pb_c,
                "w1": W1,
                "b1": b1v,
                "w2": W2v,
                "b2": b2v,
            }
        )

    res = run_bass_kernel_spmd(nc, in_maps, list(range(N_CORES)))
    global LAST_RESULTS
    LAST_RESULTS = res

    logits = np.empty(n_pairs, dtype=np.float32)
    for c in range(N_CORES):
        lo = c * per_core
        hi = min(lo + per_core, n_pairs)
        logits[lo:hi] = res.results[c]["logits_out"][: hi - lo]

    # BCE-with-logits, mean reduction (stable form), as part of unsharding
    l64 = logits.astype(np.float64)
    y64 = lab_f.astype(np.float64)
    terms = np.maximum(l64, 0.0) - l64 * y64 + np.log1p(np.exp(-np.abs(l64)))
    loss = np.float32(terms.mean())
    return loss, logits


# revision 21
# speedup vs baseline: 1.0667x; 1.0201x over previous
"""Trainium2 Bass kernel for EntityConsolidationHead (pair MLP + BCE loss).

reference math (per pair p):
    a = emb[pair_a[p]]; b = emb[pair_b[p]]              # [768]
    feats = [a, b, |a-b|, a*b]                          # [3072]
    h = gelu_exact(feats @ W1 + b1)                     # [768]
    logit = h @ W2 + b2
    loss = mean(relu(logit) - logit*y + softplus(-|logit|))

Sharding: data-parallel over pairs across 8 NeuronCores; node_embeddings
and MLP weights replicated. Per-core partial loss sums are reduced on host.
"""

import sys

sys.path.insert(0, "/opt/trn_rl_repo")

import numpy as np

N_CORES = 8
H = 768
H4 = 4 * H  # 3072
P = 128  # SBUF partitions / pair-tile size
KC = H4 // P  # 24 contraction chunks of 128
N_NODES = 50000
N_PAIRS = 100000

_CACHE = {}


def _build_program(n_nodes: int, n_tiles: int):
    """Build the SPMD Bass program for one core handling n_tiles*128 pairs."""
    import concourse.bass as bass
    import concourse.mybir as mybir
    import concourse.tile as tile
    from concourse.masks import make_identity
    from concourse.vector_clock import ScopedClock

    TC = tile.TileContext
    _ = ScopedClock  # retained import

    f32 = mybir.dt.float32
    bf16 = mybir.dt.bfloat16
    i32 = mybir.dt.int32
    NP = n_tiles * P

    nc = bass.Bass()
    emb = nc.declare_dram_parameter("emb", [n_nodes, H], f32, isOutput=False)
    pa = nc.declare_dram_parameter("pa", [P, n_tiles], i32, isOutput=False)
    pb = nc.declare_dram_parameter("pb", [P, n_tiles], i32, isOutput=False)
    w1 = nc.declare_dram_parameter("w1", [H4, H], f32, isOutput=False)
    b1 = nc.declare_dram_parameter("b1", [H], f32, isOutput=False)
    w2 = nc.declare_dram_parameter("w2", [H], f32, isOutput=False)
    b2 = nc.declare_dram_parameter("b2", [1], f32, isOutput=False)
    logits_out = nc.declare_dram_parameter("logits_out", [NP], f32, isOutput=True)

    AF = mybir.ActivationFunctionType
    OP = mybir.AluOpType

    with TC(nc) as tc:
        with (
            tc.tile_pool(name="const", bufs=1) as cpool,
            tc.tile_pool(name="work", bufs=1) as wpool,
            tc.tile_pool(name="io", bufs=4) as iopool,
            tc.tile_pool(name="mid", bufs=3) as midpool,
            tc.tile_pool(name="psum_t", bufs=2, space="PSUM") as tpsum,
            tc.tile_pool(name="psum_h", bufs=4, space="PSUM") as hpsum,
        ):
            # ---- constants / weights resident in SBUF ----
            ident = cpool.tile([P, P], f32, tag="ident")
            make_identity(nc, ident[:])
            identb = cpool.tile([P, P], bf16, tag="identb")
            make_identity(nc, identb[:])
            w1_sb = cpool.tile([P, KC * H], bf16, tag="w1sb")  # chunk k at [:, k*H:(k+1)*H]
            w1_view = w1.rearrange("(k p) n -> k p n", p=P)
            for k in range(KC):
                nc.gpsimd.dma_start(
                    out=w1_sb[:, k * H : (k + 1) * H], in_=w1_view[k]
                )
            b1_rep = cpool.tile([P, H], f32, tag="b1rep")
            nc.sync.dma_start(out=b1_rep[:], in_=b1[None, :].to_broadcast([P, H]))
            w2_rep = cpool.tile([P, H], f32, tag="w2rep")
            nc.sync.dma_start(out=w2_rep[:], in_=w2[None, :].to_broadcast([P, H]))
            b2_rep = cpool.tile([P, 1], f32, tag="b2rep")
            nc.sync.dma_start(out=b2_rep[:], in_=b2[None, :].to_broadcast([P, 1]))
            logit_sb = wpool.tile([P, n_tiles], f32, tag="logit")
            ia_all = cpool.tile([P, n_tiles], i32, tag="iaall")
            nc.sync.dma_start(out=ia_all[:], in_=pa[:])
            ib_all = cpool.tile([P, n_tiles], i32, tag="iball")
            nc.sync.dma_start(out=ib_all[:], in_=pb[:])

            # ---- main loop over pair tiles ----
            for t in range(n_tiles):
                ga = iopool.tile([P, H], bf16, tag="ga")
                gb = iopool.tile([P, H], bf16, tag="gb")
                nc.gpsimd.indirect_dma_start(
                    out=ga[:], out_offset=None, in_=emb[:],
                    in_offset=bass.IndirectOffsetOnAxis(ap=ia_all[:, t : t + 1], axis=0),
                )
                nc.gpsimd.indirect_dma_start(
                    out=gb[:], out_offset=None, in_=emb[:],
                    in_offset=bass.IndirectOffsetOnAxis(ap=ib_all[:, t : t + 1], axis=0),
                )

                # PE-transpose (bf16, 1 cyc/row) to feature-major [768, 128]
                aT = midpool.tile([P, H], bf16, tag="aT")
                bT = midpool.tile([P, H], bf16, tag="bT")
                for src, dstT, tg in ((ga, aT, "tpa"), (gb, bT, "tpb")):
                    for half in range(2):
                        tp = tpsum.tile([P, 3 * P], bf16, tag=tg, space="PSUM")
                        for c in range(3):
                            cc = half * 3 + c
                            nc.tensor.transpose(
                                out=tp[:, c * P : (c + 1) * P],
                                in_=src[:, cc * P : (cc + 1) * P],
                                identity=identb[:],
                            )
                        nc.scalar.copy(
                            out=dstT[:, half * 3 * P : (half + 1) * 3 * P], in_=tp[:]
                        )

                dif = midpool.tile([P, H], bf16, tag="dif")
                prd = midpool.tile([P, H], bf16, tag="prd")
                nc.vector.tensor_sub(out=dif[:], in0=aT[:], in1=bT[:])
                nc.scalar.activation(out=dif[:], in_=dif[:], func=AF.Abs)
                nc.vector.tensor_mul(out=prd[:], in0=aT[:], in1=bT[:])

                # h = gelu(F @ W1 + b1), pairs-major [128, 768]
                h_sb = midpool.tile([P, H], f32, tag="h")
                for half in range(2):
                    ph = hpsum.tile([P, 384], f32, tag="ph", space="PSUM")
                    nsl = slice(half * 384, (half + 1) * 384)
                    for k in range(KC):
                        if k < 6:
                            lhs = aT[:, k * P : (k + 1) * P]
                        elif k < 12:
                            lhs = bT[:, (k - 6) * P : (k - 5) * P]
                        elif k < 18:
                            lhs = dif[:, (k - 12) * P : (k - 11) * P]
                        else:
                            lhs = prd[:, (k - 18) * P : (k - 17) * P]
                        nc.tensor.matmul(
                            out=ph[:],
                            lhsT=lhs,
                            rhs=w1_sb[:, k * H + half * 384 : k * H + half * 384 + 384],
                            start=(k == 0),
                            stop=(k == KC - 1),
                        )
                    nc.vector.tensor_add(
                        out=h_sb[:, nsl], in0=ph[:], in1=b1_rep[:, nsl]
                    )
                    nc.scalar.activation(
                        out=h_sb[:, nsl], in_=h_sb[:, nsl], func=AF.Gelu
                    )

                # logit[p] = sum_f h[p,f] * w2[f]  (fused mul + free-dim sum)
                trash = midpool.tile([P, H], f32, tag="trash")
                nc.vector.scalar_tensor_tensor(
                    out=trash[:],
                    in0=h_sb[:],
                    scalar=0.0,
                    in1=w2_rep[:],
                    op0=OP.add,
                    op1=OP.mult,
                    accum_out=logit_sb[:, t : t + 1],
                )

            # ---- epilogue: +b2, logits out ----
            nc.vector.tensor_scalar(
                out=logit_sb[:], in0=logit_sb[:], scalar1=b2_rep[:, :1],
                scalar2=None, op0=OP.add,
            )
            # logits out: transpose [128, T] -> [T, 128] then contiguous DMA
            ltp = tpsum.tile([P, P], f32, tag="tpa", space="PSUM")
            nc.tensor.transpose(out=ltp[:n_tiles, :], in_=logit_sb[:], identity=ident[:])
            lo_sb = wpool.tile([P, P], f32, tag="losb")
            nc.scalar.copy(out=lo_sb[:n_tiles, :], in_=ltp[:n_tiles, :])
            nc.sync.dma_start(
                out=logits_out.rearrange("(t l) -> t l", l=P), in_=lo_sb[:n_tiles, :]
            )

    _split_multi_waits(nc, mybir)
    return nc


def _split_multi_waits(nc, mybir):
    """This walrus build allows at most one sync-wait per instruction:
    hoist extra waits onto same-engine NOPs placed just before."""
    uid = 0
    for bb in nc.main_func.blocks:
        new_list = []
        for inst in bb.instructions:
            si = inst.sync_info
            if si is not None and si.on_wait and len(si.on_wait) > 1:
                waits = list(si.on_wait)
                for sw in waits[:-1]:
                    uid += 1
                    new_list.append(
                        mybir.InstNoOp(
                            name=f"I-wsplit-{uid}",
                            sync_info=mybir.SyncInfo(on_wait=[sw], on_update=[]),
                            bass_nofuse=True,
                            engine=inst.engine,
                        )
                    )
                si.on_wait = waits[-1:]
            new_list.append(inst)
        bb.instructions[:] = new_list


def kernel(node_embeddings, pair_a, pair_b, labels, W1, b1, W2, b2):
    from concourse.bass_utils import run_bass_kernel_spmd

    node_embeddings = np.ascontiguousarray(np.asarray(node_embeddings, dtype=np.float32))
    pair_a = np.asarray(pair_a).astype(np.int32)
    pair_b = np.asarray(pair_b).astype(np.int32)
    labels_in = np.asarray(labels)
    W1 = np.ascontiguousarray(np.asarray(W1, dtype=np.float32))
    b1v = np.asarray(b1, dtype=np.float32).reshape(-1)
    W2v = np.asarray(W2, dtype=np.float32).reshape(-1)
    b2v = np.asarray(b2, dtype=np.float32).reshape(-1)

    n_nodes = node_embeddings.shape[0]
    n_pairs = pair_a.shape[0]
    per_core = -(-n_pairs // N_CORES)
    n_tiles = -(-per_core // P)
    npc = n_tiles * P  # padded pairs per core

    key = (n_nodes, n_tiles)
    if key not in _CACHE:
        _CACHE[key] = _build_program(n_nodes, n_tiles)
    nc = _CACHE[key]

    lab_f = labels_in.astype(np.float32)
    in_maps = []
    for c in range(N_CORES):
        lo = c * per_core
        hi = min(lo + per_core, n_pairs)
        n = hi - lo
        pa_c = np.zeros(npc, dtype=np.int32)
        pb_c = np.zeros(npc, dtype=np.int32)
        pa_c[:n] = pair_a[lo:hi]
        pb_c[:n] = pair_b[lo:hi]
        pa_c = np.ascontiguousarray(pa_c.reshape(n_tiles, P).T)
        pb_c = np.ascontiguousarray(pb_c.reshape(n_tiles, P).T)
        in_maps.append(
            {
                "emb": node_embeddings,
                "pa": pa_c,
                "pb": pb_c,
                "w1": W1,
                "b1": b1v,
                "w2": W2v,
                "b2": b2v,
            }
        )

    try:
        res = run_bass_kernel_spmd(nc, in_maps, list(range(N_CORES)))
    except Exception:
        # transient device wedge (e.g. NRT_EXEC_UNIT_UNRECOVERABLE) — retry once
        import time as _time

        _time.sleep(2.0)
        res = run_bass_kernel_spmd(nc, in_maps, list(range(N_CORES)))
    global LAST_RESULTS
    LAST_RESULTS = res

    logits = np.empty(n_pairs, dtype=np.float32)
    for c in range(N_CORES):
        lo = c * per_core
        hi = min(lo + per_core, n_pairs)
        logits[lo:hi] = res.results[c]["logits_out"][: hi - lo]

    # BCE-with-logits, mean reduction (stable form), as part of unsharding
    l64 = logits.astype(np.float64)
    y64 = lab_f.astype(np.float64)
    terms = np.maximum(l64, 0.0) - l64 * y64 + np.log1p(np.exp(-np.abs(l64)))
    loss = np.float32(terms.mean())
    return loss, logits


# revision 22
# speedup vs baseline: 1.0738x; 1.0066x over previous
"""Trainium2 Bass kernel for EntityConsolidationHead (pair MLP + BCE loss).

reference math (per pair p):
    a = emb[pair_a[p]]; b = emb[pair_b[p]]              # [768]
    feats = [a, b, |a-b|, a*b]                          # [3072]
    h = gelu_exact(feats @ W1 + b1)                     # [768]
    logit = h @ W2 + b2
    loss = mean(relu(logit) - logit*y + softplus(-|logit|))

Sharding: data-parallel over pairs across 8 NeuronCores; node_embeddings
and MLP weights replicated. Per-core partial loss sums are reduced on host.
"""

import sys

sys.path.insert(0, "/opt/trn_rl_repo")

import numpy as np

N_CORES = 8
H = 768
H4 = 4 * H  # 3072
P = 128  # SBUF partitions / pair-tile size
KC = H4 // P  # 24 contraction chunks of 128
N_NODES = 50000
N_PAIRS = 100000

_CACHE = {}


def _build_program(n_nodes: int, n_tiles: int):
    """Build the SPMD Bass program for one core handling n_tiles*128 pairs."""
    import concourse.bass as bass
    import concourse.mybir as mybir
    import concourse.tile as tile
    from concourse.masks import make_identity
    from concourse.vector_clock import ScopedClock

    TC = tile.TileContext
    _ = ScopedClock  # retained import

    f32 = mybir.dt.float32
    bf16 = mybir.dt.bfloat16
    i32 = mybir.dt.int32
    NP = n_tiles * P

    nc = bass.Bass()
    emb = nc.declare_dram_parameter("emb", [n_nodes, H], f32, isOutput=False)
    pa = nc.declare_dram_parameter("pa", [P, n_tiles], i32, isOutput=False)
    pb = nc.declare_dram_parameter("pb", [P, n_tiles], i32, isOutput=False)
    w1 = nc.declare_dram_parameter("w1", [H4, H], f32, isOutput=False)
    b1 = nc.declare_dram_parameter("b1", [H], f32, isOutput=False)
    w2 = nc.declare_dram_parameter("w2", [H], f32, isOutput=False)
    b2 = nc.declare_dram_parameter("b2", [1], f32, isOutput=False)
    logits_out = nc.declare_dram_parameter("logits_out", [NP], f32, isOutput=True)

    AF = mybir.ActivationFunctionType
    OP = mybir.AluOpType

    with TC(nc) as tc:
        with (
            tc.tile_pool(name="const", bufs=1) as cpool,
            tc.tile_pool(name="work", bufs=1) as wpool,
            tc.tile_pool(name="io", bufs=4) as iopool,
            tc.tile_pool(name="mid", bufs=3) as midpool,
            tc.tile_pool(name="psum_t", bufs=2, space="PSUM") as tpsum,
            tc.tile_pool(name="psum_h", bufs=4, space="PSUM") as hpsum,
        ):
            # ---- constants / weights resident in SBUF ----
            ident = cpool.tile([P, P], f32, tag="ident")
            make_identity(nc, ident[:])
            identb = cpool.tile([P, P], bf16, tag="identb")
            make_identity(nc, identb[:])
            b1_rep = cpool.tile([P, H], f32, tag="b1rep")
            nc.sync.dma_start(out=b1_rep[:], in_=b1[None, :].to_broadcast([P, H]))
            w2_rep = cpool.tile([P, H], f32, tag="w2rep")
            nc.sync.dma_start(out=w2_rep[:], in_=w2[None, :].to_broadcast([P, H]))
            b2_rep = cpool.tile([P, 1], f32, tag="b2rep")
            nc.sync.dma_start(out=b2_rep[:], in_=b2[None, :].to_broadcast([P, 1]))
            logit_sb = wpool.tile([P, n_tiles], f32, tag="logit")
            ia_all = cpool.tile([P, n_tiles], i32, tag="iaall")
            nc.sync.dma_start(out=ia_all[:], in_=pa[:])
            ib_all = cpool.tile([P, n_tiles], i32, tag="iball")
            nc.sync.dma_start(out=ib_all[:], in_=pb[:])

            # first two tiles' gathers go on the SWDGE queue BEFORE the W1
            # chunk loads so PE work can start ~25us earlier
            pre_g = []
            for t in range(min(2, n_tiles)):
                ga = iopool.tile([P, H], bf16, tag="ga")
                gb = iopool.tile([P, H], bf16, tag="gb")
                nc.gpsimd.indirect_dma_start(
                    out=ga[:], out_offset=None, in_=emb[:],
                    in_offset=bass.IndirectOffsetOnAxis(ap=ia_all[:, t : t + 1], axis=0),
                )
                nc.gpsimd.indirect_dma_start(
                    out=gb[:], out_offset=None, in_=emb[:],
                    in_offset=bass.IndirectOffsetOnAxis(ap=ib_all[:, t : t + 1], axis=0),
                )
                pre_g.append((ga, gb))

            w1_sb = cpool.tile([P, KC * H], bf16, tag="w1sb")  # chunk k at [:, k*H:(k+1)*H]
            w1_view = w1.rearrange("(k p) n -> k p n", p=P)
            for k in range(KC):
                nc.gpsimd.dma_start(
                    out=w1_sb[:, k * H : (k + 1) * H], in_=w1_view[k]
                )

            # ---- main loop over pair tiles ----
            for t in range(n_tiles):
                if t < len(pre_g):
                    ga, gb = pre_g[t]
                else:
                    ga = iopool.tile([P, H], bf16, tag="ga")
                    gb = iopool.tile([P, H], bf16, tag="gb")
                    nc.gpsimd.indirect_dma_start(
                        out=ga[:], out_offset=None, in_=emb[:],
                        in_offset=bass.IndirectOffsetOnAxis(ap=ia_all[:, t : t + 1], axis=0),
                    )
                    nc.gpsimd.indirect_dma_start(
                        out=gb[:], out_offset=None, in_=emb[:],
                        in_offset=bass.IndirectOffsetOnAxis(ap=ib_all[:, t : t + 1], axis=0),
                    )

                # PE-transpose (bf16, 1 cyc/row) to feature-major [768, 128]
                aT = midpool.tile([P, H], bf16, tag="aT")
                bT = midpool.tile([P, H], bf16, tag="bT")
                for src, dstT, tg in ((ga, aT, "tpa"), (gb, bT, "tpb")):
                    for half in range(2):
                        tp = tpsum.tile([P, 3 * P], bf16, tag=tg, space="PSUM")
                        for c in range(3):
                            cc = half * 3 + c
                            nc.tensor.transpose(
                                out=tp[:, c * P : (c + 1) * P],
                                in_=src[:, cc * P : (cc + 1) * P],
                                identity=identb[:],
                            )
                        nc.scalar.copy(
                            out=dstT[:, half * 3 * P : (half + 1) * 3 * P], in_=tp[:]
                        )

                dif = midpool.tile([P, H], bf16, tag="dif")
                prd = midpool.tile([P, H], bf16, tag="prd")
                nc.vector.tensor_sub(out=dif[:], in0=aT[:], in1=bT[:])
                nc.scalar.activation(out=dif[:], in_=dif[:], func=AF.Abs)
                nc.vector.tensor_mul(out=prd[:], in0=aT[:], in1=bT[:])

                # h = gelu(F @ W1 + b1), pairs-major [128, 768]
                h_sb = midpool.tile([P, H], f32, tag="h")
                for half in range(2):
                    ph = hpsum.tile([P, 384], f32, tag="ph", space="PSUM")
                    nsl = slice(half * 384, (half + 1) * 384)
                    for k in range(KC):
                        if k < 6:
                            lhs = aT[:, k * P : (k + 1) * P]
                        elif k < 12:
                            lhs = bT[:, (k - 6) * P : (k - 5) * P]
                        elif k < 18:
                            lhs = dif[:, (k - 12) * P : (k - 11) * P]
                        else:
                            lhs = prd[:, (k - 18) * P : (k - 17) * P]
                        nc.tensor.matmul(
                            out=ph[:],
                            lhsT=lhs,
                            rhs=w1_sb[:, k * H + half * 384 : k * H + half * 384 + 384],
                            start=(k == 0),
                            stop=(k == KC - 1),
                        )
                    nc.vector.tensor_add(
                        out=h_sb[:, nsl], in0=ph[:], in1=b1_rep[:, nsl]
                    )
                    nc.scalar.activation(
                        out=h_sb[:, nsl], in_=h_sb[:, nsl], func=AF.Gelu
                    )

                # logit[p] = sum_f h[p,f] * w2[f]  (fused mul + free-dim sum)
                trash = midpool.tile([P, H], f32, tag="trash")
                nc.vector.scalar_tensor_tensor(
                    out=trash[:],
                    in0=h_sb[:],
                    scalar=0.0,
                    in1=w2_rep[:],
                    op0=OP.add,
                    op1=OP.mult,
                    accum_out=logit_sb[:, t : t + 1],
                )

            # ---- epilogue: +b2, logits out ----
            nc.vector.tensor_scalar(
                out=logit_sb[:], in0=logit_sb[:], scalar1=b2_rep[:, :1],
                scalar2=None, op0=OP.add,
            )
            # logits out: transpose [128, T] -> [T, 128] then contiguous DMA
            ltp = tpsum.tile([P, P], f32, tag="tpa", space="PSUM")
            nc.tensor.transpose(out=ltp[:n_tiles, :], in_=logit_sb[:], identity=ident[:])
            lo_sb = wpool.tile([P, P], f32, tag="losb")
            nc.scalar.copy(out=lo_sb[:n_tiles, :], in_=ltp[:n_tiles, :])
            nc.sync.dma_start(
                out=logits_out.rearrange("(t l) -> t l", l=P), in_=lo_sb[:n_tiles, :]
            )

    _split_multi_waits(nc, mybir)
    return nc


def _split_multi_waits(nc, mybir):
    """This walrus build allows at most one sync-wait per instruction:
    hoist extra waits onto same-engine NOPs placed just before."""
    uid = 0
    for bb in nc.main_func.blocks:
        new_list = []
        for inst in bb.instructions:
            si = inst.sync_info
            if si is not None and si.on_wait and len(si.on_wait) > 1:
                waits = list(si.on_wait)
                for sw in waits[:-1]:
                    uid += 1
                    new_list.append(
                        mybir.InstNoOp(
                            name=f"I-wsplit-{uid}",
                            sync_info=mybir.SyncInfo(on_wait=[sw], on_update=[]),
                            bass_nofuse=True,
                            engine=inst.engine,
                        )
                    )
                si.on_wait = waits[-1:]
            new_list.append(inst)
        bb.instructions[:] = new_list


def kernel(node_embeddings, pair_a, pair_b, labels, W1, b1, W2, b2):
    from concourse.bass_utils import run_bass_kernel_spmd

    node_embeddings = np.ascontiguousarray(np.asarray(node_embeddings, dtype=np.float32))
    pair_a = np.asarray(pair_a).astype(np.int32)
    pair_b = np.asarray(pair_b).astype(np.int32)
    labels_in = np.asarray(labels)
    W1 = np.ascontiguousarray(np.asarray(W1, dtype=np.float32))
    b1v = np.asarray(b1, dtype=np.float32).reshape(-1)
    W2v = np.asarray(W2, dtype=np.float32).reshape(-1)
    b2v = np.asarray(b2, dtype=np.float32).reshape(-1)

    n_nodes = node_embeddings.shape[0]
    n_pairs = pair_a.shape[0]
    per_core = -(-n_pairs // N_CORES)
    n_tiles = -(-per_core // P)
    npc = n_tiles * P  # padded pairs per core

    key = (n_nodes, n_tiles)
    if key not in _CACHE:
        _CACHE[key] = _build_program(n_nodes, n_tiles)
    nc = _CACHE[key]

    lab_f = labels_in.astype(np.float32)
    in_maps = []
    for c in range(N_CORES):
        lo = c * per_core
        hi = min(lo + per_core, n_pairs)
        n = hi - lo
        pa_c = np.zeros(npc, dtype=np.int32)
        pb_c = np.zeros(npc, dtype=np.int32)
        pa_c[:n] = pair_a[lo:hi]
        pb_c[:n] = pair_b[lo:hi]
        pa_c = np.ascontiguousarray(pa_c.reshape(n_tiles, P).T)
        pb_c = np.ascontiguousarray(pb_c.reshape(n_tiles, P).T)
        in_maps.append(
            {
                "emb": node_embeddings,
                "pa": pa_c,
                "pb": pb_c,
                "w1": W1,
                "b1": b1v,
                "w2": W2v,
                "b2": b2v,
            }
        )

    try:
        res = run_bass_kernel_spmd(nc, in_maps, list(range(N_CORES)))
    except Exception:
        # transient device wedge (e.g. NRT_EXEC_UNIT_UNRECOVERABLE) — retry once
        import time as _time

        _time.sleep(2.0)
        res = run_bass_kernel_spmd(nc, in_maps, list(range(N_CORES)))
    global LAST_RESULTS
    LAST_RESULTS = res

    logits = np.empty(n_pairs, dtype=np.float32)
    for c in range(N_CORES):
        lo = c * per_core
        hi = min(lo + per_core, n_pairs)
        logits[lo:hi] = res.results[c]["logits_out"][: hi - lo]

    # BCE-with-logits, mean reduction (stable form), as part of unsharding
    l64 = logits.astype(np.float64)
    y64 = lab_f.astype(np.float64)
    terms = np.maximum(l64, 0.0) - l64 * y64 + np.log1p(np.exp(-np.abs(l64)))
    loss = np.float32(terms.mean())
    return loss, logits


# revision 23
# speedup vs baseline: 1.0782x; 1.0041x over previous
"""Trainium2 Bass kernel for EntityConsolidationHead (pair MLP + BCE loss).

reference math (per pair p):
    a = emb[pair_a[p]]; b = emb[pair_b[p]]              # [768]
    feats = [a, b, |a-b|, a*b]                          # [3072]
    h = gelu_exact(feats @ W1 + b1)                     # [768]
    logit = h @ W2 + b2
    loss = mean(relu(logit) - logit*y + softplus(-|logit|))

Sharding: data-parallel over pairs across 8 NeuronCores; node_embeddings
and MLP weights replicated. Per-core partial loss sums are reduced on host.
"""

import sys

sys.path.insert(0, "/opt/trn_rl_repo")

import numpy as np

N_CORES = 8
H = 768
H4 = 4 * H  # 3072
P = 128  # SBUF partitions / pair-tile size
KC = H4 // P  # 24 contraction chunks of 128
N_NODES = 50000
N_PAIRS = 100000

_CACHE = {}


def _build_program(n_nodes: int, n_tiles: int):
    """Build the SPMD Bass program for one core handling n_tiles*128 pairs."""
    import concourse.bass as bass
    import concourse.mybir as mybir
    import concourse.tile as tile
    from concourse.masks import make_identity
    from concourse.vector_clock import ScopedClock

    TC = tile.TileContext
    _ = ScopedClock  # retained import

    f32 = mybir.dt.float32
    bf16 = mybir.dt.bfloat16
    i32 = mybir.dt.int32
    NP = n_tiles * P

    nc = bass.Bass()
    emb = nc.declare_dram_parameter("emb", [n_nodes, H], f32, isOutput=False)
    pa = nc.declare_dram_parameter("pa", [P, n_tiles], i32, isOutput=False)
    pb = nc.declare_dram_parameter("pb", [P, n_tiles], i32, isOutput=False)
    w1 = nc.declare_dram_parameter("w1", [H4, H], bf16, isOutput=False)
    b1 = nc.declare_dram_parameter("b1", [H], f32, isOutput=False)
    w2 = nc.declare_dram_parameter("w2", [H], f32, isOutput=False)
    b2 = nc.declare_dram_parameter("b2", [1], f32, isOutput=False)
    logits_out = nc.declare_dram_parameter("logits_out", [NP], f32, isOutput=True)

    AF = mybir.ActivationFunctionType
    OP = mybir.AluOpType

    with TC(nc) as tc:
        with (
            tc.tile_pool(name="const", bufs=1) as cpool,
            tc.tile_pool(name="work", bufs=1) as wpool,
            tc.tile_pool(name="io", bufs=4) as iopool,
            tc.tile_pool(name="mid", bufs=3) as midpool,
            tc.tile_pool(name="psum_t", bufs=2, space="PSUM") as tpsum,
            tc.tile_pool(name="psum_h", bufs=4, space="PSUM") as hpsum,
        ):
            # ---- constants / weights resident in SBUF ----
            b1_rep = cpool.tile([P, H], f32, tag="b1rep")
            nc.sync.dma_start(out=b1_rep[:], in_=b1[None, :].to_broadcast([P, H]))
            w2_rep = cpool.tile([P, H], f32, tag="w2rep")
            nc.sync.dma_start(out=w2_rep[:], in_=w2[None, :].to_broadcast([P, H]))
            b2_rep = cpool.tile([P, 1], f32, tag="b2rep")
            nc.sync.dma_start(out=b2_rep[:], in_=b2[None, :].to_broadcast([P, 1]))
            logit_sb = wpool.tile([P, n_tiles], f32, tag="logit")
            ia_all = cpool.tile([P, n_tiles], i32, tag="iaall")
            nc.sync.dma_start(out=ia_all[:], in_=pa[:])
            ib_all = cpool.tile([P, n_tiles], i32, tag="iball")
            nc.sync.dma_start(out=ib_all[:], in_=pb[:])

            # first two tiles' gathers go on the SWDGE queue BEFORE the W1
            # chunk loads so PE work can start ~25us earlier
            pre_g = []
            for t in range(min(2, n_tiles)):
                ga = iopool.tile([P, H], bf16, tag="ga")
                gb = iopool.tile([P, H], bf16, tag="gb")
                nc.gpsimd.indirect_dma_start(
                    out=ga[:], out_offset=None, in_=emb[:],
                    in_offset=bass.IndirectOffsetOnAxis(ap=ia_all[:, t : t + 1], axis=0),
                )
                nc.gpsimd.indirect_dma_start(
                    out=gb[:], out_offset=None, in_=emb[:],
                    in_offset=bass.IndirectOffsetOnAxis(ap=ib_all[:, t : t + 1], axis=0),
                )
                pre_g.append((ga, gb))

            ident = cpool.tile([P, P], f32, tag="ident")
            make_identity(nc, ident[:])
            identb = cpool.tile([P, P], bf16, tag="identb")
            make_identity(nc, identb[:])

            # W1 (host-cast to bf16) via HWDGE, 6-chunk blocks, in k order
            w1_sb = cpool.tile([P, KC * H], bf16, tag="w1sb")  # chunk k at [:, k*H:(k+1)*H]
            w1_view = w1.rearrange("(k p) n -> p k n", p=P)
            for kb in range(0, KC, 6):
                nc.sync.dma_start(
                    out=w1_sb[:, kb * H : (kb + 6) * H],
                    in_=w1_view[:, kb : kb + 6, :],
                )

            # ---- main loop over pair tiles ----
            for t in range(n_tiles):
                if t < len(pre_g):
                    ga, gb = pre_g[t]
                else:
                    ga = iopool.tile([P, H], bf16, tag="ga")
                    gb = iopool.tile([P, H], bf16, tag="gb")
                    nc.gpsimd.indirect_dma_start(
                        out=ga[:], out_offset=None, in_=emb[:],
                        in_offset=bass.IndirectOffsetOnAxis(ap=ia_all[:, t : t + 1], axis=0),
                    )
                    nc.gpsimd.indirect_dma_start(
                        out=gb[:], out_offset=None, in_=emb[:],
                        in_offset=bass.IndirectOffsetOnAxis(ap=ib_all[:, t : t + 1], axis=0),
                    )

                # PE-transpose (bf16, 1 cyc/row) to feature-major [768, 128]
                aT = midpool.tile([P, H], bf16, tag="aT")
                bT = midpool.tile([P, H], bf16, tag="bT")
                for src, dstT, tg in ((ga, aT, "tpa"), (gb, bT, "tpb")):
                    for half in range(2):
                        tp = tpsum.tile([P, 3 * P], bf16, tag=tg, space="PSUM")
                        for c in range(3):
                            cc = half * 3 + c
                            nc.tensor.transpose(
                                out=tp[:, c * P : (c + 1) * P],
                                in_=src[:, cc * P : (cc + 1) * P],
                                identity=identb[:],
                            )
                        nc.scalar.copy(
                            out=dstT[:, half * 3 * P : (half + 1) * 3 * P], in_=tp[:]
                        )

                dif = midpool.tile([P, H], bf16, tag="dif")
                prd = midpool.tile([P, H], bf16, tag="prd")
                nc.vector.tensor_sub(out=dif[:], in0=aT[:], in1=bT[:])
                nc.scalar.activation(out=dif[:], in_=dif[:], func=AF.Abs)
                nc.vector.tensor_mul(out=prd[:], in0=aT[:], in1=bT[:])

                # h = gelu(F @ W1 + b1), pairs-major [128, 768]
                h_sb = midpool.tile([P, H], f32, tag="h")
                for half in range(2):
                    ph = hpsum.tile([P, 384], f32, tag="ph", space="PSUM")
                    nsl = slice(half * 384, (half + 1) * 384)
                    for k in range(KC):
                        if k < 6:
                            lhs = aT[:, k * P : (k + 1) * P]
                        elif k < 12:
                            lhs = bT[:, (k - 6) * P : (k - 5) * P]
                        elif k < 18:
                            lhs = dif[:, (k - 12) * P : (k - 11) * P]
                        else:
                            lhs = prd[:, (k - 18) * P : (k - 17) * P]
                        nc.tensor.matmul(
                            out=ph[:],
                            lhsT=lhs,
                            rhs=w1_sb[:, k * H + half * 384 : k * H + half * 384 + 384],
                            start=(k == 0),
                            stop=(k == KC - 1),
                        )
                    nc.vector.tensor_add(
                        out=h_sb[:, nsl], in0=ph[:], in1=b1_rep[:, nsl]
                    )
                    nc.scalar.activation(
                        out=h_sb[:, nsl], in_=h_sb[:, nsl], func=AF.Gelu
                    )

                # logit[p] = sum_f h[p,f] * w2[f]  (fused mul + free-dim sum)
                trash = midpool.tile([P, H], f32, tag="trash")
                nc.vector.scalar_tensor_tensor(
                    out=trash[:],
                    in0=h_sb[:],
                    scalar=0.0,
                    in1=w2_rep[:],
                    op0=OP.add,
                    op1=OP.mult,
                    accum_out=logit_sb[:, t : t + 1],
                )

            # ---- epilogue: +b2, logits out ----
            nc.vector.tensor_scalar(
                out=logit_sb[:], in0=logit_sb[:], scalar1=b2_rep[:, :1],
                scalar2=None, op0=OP.add,
            )
            # logits out: transpose [128, T] -> [T, 128] then contiguous DMA
            ltp = tpsum.tile([P, P], f32, tag="tpa", space="PSUM")
            nc.tensor.transpose(out=ltp[:n_tiles, :], in_=logit_sb[:], identity=ident[:])
            lo_sb = wpool.tile([P, P], f32, tag="losb")
            nc.scalar.copy(out=lo_sb[:n_tiles, :], in_=ltp[:n_tiles, :])
            nc.sync.dma_start(
                out=logits_out.rearrange("(t l) -> t l", l=P), in_=lo_sb[:n_tiles, :]
            )

    _split_multi_waits(nc, mybir)
    return nc


def _split_multi_waits(nc, mybir):
    """This walrus build allows at most one sync-wait per instruction:
    hoist extra waits onto same-engine NOPs placed just before."""
    uid = 0
    for bb in nc.main_func.blocks:
        new_list = []
        for inst in bb.instructions:
            si = inst.sync_info
            if si is not None and si.on_wait and len(si.on_wait) > 1:
                waits = list(si.on_wait)
                for sw in waits[:-1]:
                    uid += 1
                    new_list.append(
                        mybir.InstNoOp(
                            name=f"I-wsplit-{uid}",
                            sync_info=mybir.SyncInfo(on_wait=[sw], on_update=[]),
                            bass_nofuse=True,
                            engine=inst.engine,
                        )
                    )
                si.on_wait = waits[-1:]
            new_list.append(inst)
        bb.instructions[:] = new_list


def kernel(node_embeddings, pair_a, pair_b, labels, W1, b1, W2, b2):
    from concourse.bass_utils import run_bass_kernel_spmd

    node_embeddings = np.ascontiguousarray(np.asarray(node_embeddings, dtype=np.float32))
    pair_a = np.asarray(pair_a).astype(np.int32)
    pair_b = np.asarray(pair_b).astype(np.int32)
    labels_in = np.asarray(labels)
    import ml_dtypes

    W1 = np.ascontiguousarray(
        np.asarray(W1, dtype=np.float32).astype(ml_dtypes.bfloat16)
    )
    b1v = np.asarray(b1, dtype=np.float32).reshape(-1)
    W2v = np.asarray(W2, dtype=np.float32).reshape(-1)
    b2v = np.asarray(b2, dtype=np.float32).reshape(-1)

    n_nodes = node_embeddings.shape[0]
    n_pairs = pair_a.shape[0]
    per_core = -(-n_pairs // N_CORES)
    n_tiles = -(-per_core // P)
    npc = n_tiles * P  # padded pairs per core

    key = (n_nodes, n_tiles)
    if key not in _CACHE:
        _CACHE[key] = _build_program(n_nodes, n_tiles)
    nc = _CACHE[key]

    lab_f = labels_in.astype(np.float32)
    in_maps = []
    for c in range(N_CORES):
        lo = c * per_core
        hi = min(lo + per_core, n_pairs)
        n = hi - lo
        pa_c = np.zeros(npc, dtype=np.int32)
        pb_c = np.zeros(npc, dtype=np.int32)
        pa_c[:n] = pair_a[lo:hi]
        pb_c[:n] = pair_b[lo:hi]
        pa_c = np.ascontiguousarray(pa_c.reshape(n_tiles, P).T)
        pb_c = np.ascontiguousarray(pb_c.reshape(n_tiles, P).T)
        in_maps.append(
            {
                "emb": node_embeddings,
                "pa": pa_c,
                "pb": pb_c,
                "w1": W1,
                "b1": b1v,
                "w2": W2v,
                "b2": b2v,
            }
        )

    try:
        res = run_bass_kernel_spmd(nc, in_maps, list(range(N_CORES)))
    except Exception:
        # transient device wedge (e.g. NRT_EXEC_UNIT_UNRECOVERABLE) — retry once
        import time as _time

        _time.sleep(2.0)
        res = run_bass_kernel_spmd(nc, in_maps, list(range(N_CORES)))
    global LAST_RESULTS
    LAST_RESULTS = res

    logits = np.empty(n_pairs, dtype=np.float32)
    for c in range(N_CORES):
        lo = c * per_core
        hi = min(lo + per_core, n_pairs)
        logits[lo:hi] = res.results[c]["logits_out"][: hi - lo]

    # BCE-with-logits, mean reduction (stable form), as part of unsharding
    l64 = logits.astype(np.float64)
    y64 = lab_f.astype(np.float64)
    terms = np.maximum(l64, 0.0) - l64 * y64 + np.log1p(np.exp(-np.abs(l64)))
    loss = np.float32(terms.mean())
    return loss, logits


# revision 24
# speedup vs baseline: 1.0798x; 1.0015x over previous
"""Trainium2 Bass kernel for EntityConsolidationHead (pair MLP + BCE loss).

reference math (per pair p):
    a = emb[pair_a[p]]; b = emb[pair_b[p]]              # [768]
    feats = [a, b, |a-b|, a*b]                          # [3072]
    h = gelu_exact(feats @ W1 + b1)                     # [768]
    logit = h @ W2 + b2
    loss = mean(relu(logit) - logit*y + softplus(-|logit|))

Sharding: data-parallel over pairs across 8 NeuronCores; node_embeddings
and MLP weights replicated. Per-core partial loss sums are reduced on host.
"""

import sys

sys.path.insert(0, "/opt/trn_rl_repo")

import numpy as np

N_CORES = 8
H = 768
H4 = 4 * H  # 3072
P = 128  # SBUF partitions / pair-tile size
KC = H4 // P  # 24 contraction chunks of 128
N_NODES = 50000
N_PAIRS = 100000

_CACHE = {}


def _build_program(n_nodes: int, n_tiles: int):
    """Build the SPMD Bass program for one core handling n_tiles*128 pairs."""
    import concourse.bass as bass
    import concourse.mybir as mybir
    import concourse.tile as tile
    from concourse.masks import make_identity
    from concourse.vector_clock import ScopedClock

    TC = tile.TileContext
    _ = ScopedClock  # retained import

    f32 = mybir.dt.float32
    bf16 = mybir.dt.bfloat16
    i32 = mybir.dt.int32
    NP = n_tiles * P

    nc = bass.Bass()
    emb = nc.declare_dram_parameter("emb", [n_nodes, H], f32, isOutput=False)
    pa = nc.declare_dram_parameter("pa", [P, n_tiles], i32, isOutput=False)
    pb = nc.declare_dram_parameter("pb", [P, n_tiles], i32, isOutput=False)
    w1 = nc.declare_dram_parameter("w1", [H4, H], bf16, isOutput=False)
    b1 = nc.declare_dram_parameter("b1", [H], f32, isOutput=False)
    w2 = nc.declare_dram_parameter("w2", [H], f32, isOutput=False)
    b2 = nc.declare_dram_parameter("b2", [1], f32, isOutput=False)
    logits_out = nc.declare_dram_parameter("logits_out", [NP], f32, isOutput=True)

    AF = mybir.ActivationFunctionType
    OP = mybir.AluOpType

    with TC(nc) as tc:
        with (
            tc.tile_pool(name="const", bufs=1) as cpool,
            tc.tile_pool(name="work", bufs=1) as wpool,
            tc.tile_pool(name="io", bufs=4) as iopool,
            tc.tile_pool(name="mid", bufs=3) as midpool,
            tc.tile_pool(name="psum_t", bufs=2, space="PSUM") as tpsum,
            tc.tile_pool(name="psum_h", bufs=4, space="PSUM") as hpsum,
        ):
            # ---- constants / weights resident in SBUF ----
            b1_rep = cpool.tile([P, H], f32, tag="b1rep")
            nc.sync.dma_start(out=b1_rep[:], in_=b1[None, :].to_broadcast([P, H]))
            w2_rep = cpool.tile([P, H], f32, tag="w2rep")
            nc.sync.dma_start(out=w2_rep[:], in_=w2[None, :].to_broadcast([P, H]))
            b2_rep = cpool.tile([P, 1], f32, tag="b2rep")
            nc.sync.dma_start(out=b2_rep[:], in_=b2[None, :].to_broadcast([P, 1]))
            logit_sb = wpool.tile([P, n_tiles], f32, tag="logit")
            ia_all = cpool.tile([P, n_tiles], i32, tag="iaall")
            nc.sync.dma_start(out=ia_all[:], in_=pa[:])
            ib_all = cpool.tile([P, n_tiles], i32, tag="iball")
            nc.sync.dma_start(out=ib_all[:], in_=pb[:])

            # first two tiles' gathers go on the SWDGE queue BEFORE the W1
            # chunk loads so PE work can start ~25us earlier
            pre_g = []
            for t in range(min(2, n_tiles)):
                ga = iopool.tile([P, H], bf16, tag="ga")
                gb = iopool.tile([P, H], bf16, tag="gb")
                nc.gpsimd.indirect_dma_start(
                    out=ga[:], out_offset=None, in_=emb[:],
                    in_offset=bass.IndirectOffsetOnAxis(ap=ia_all[:, t : t + 1], axis=0),
                )
                nc.gpsimd.indirect_dma_start(
                    out=gb[:], out_offset=None, in_=emb[:],
                    in_offset=bass.IndirectOffsetOnAxis(ap=ib_all[:, t : t + 1], axis=0),
                )
                pre_g.append((ga, gb))

            ident = cpool.tile([P, P], f32, tag="ident")
            make_identity(nc, ident[:])
            identb = cpool.tile([P, P], bf16, tag="identb")
            make_identity(nc, identb[:])

            # W1 (host-cast to bf16) via HWDGE, 6-chunk blocks, in k order
            w1_sb = cpool.tile([P, KC * H], bf16, tag="w1sb")  # chunk k at [:, k*H:(k+1)*H]
            w1_view = w1.rearrange("(k p) n -> p k n", p=P)
            for kb in range(0, KC, 6):
                nc.sync.dma_start(
                    out=w1_sb[:, kb * H : (kb + 6) * H],
                    in_=w1_view[:, kb : kb + 6, :],
                )

            # ---- main loop over pair tiles ----
            for t in range(n_tiles):
                if t < len(pre_g):
                    ga, gb = pre_g[t]
                else:
                    ga = iopool.tile([P, H], bf16, tag="ga")
                    gb = iopool.tile([P, H], bf16, tag="gb")
                    nc.gpsimd.indirect_dma_start(
                        out=ga[:], out_offset=None, in_=emb[:],
                        in_offset=bass.IndirectOffsetOnAxis(ap=ia_all[:, t : t + 1], axis=0),
                    )
                    nc.gpsimd.indirect_dma_start(
                        out=gb[:], out_offset=None, in_=emb[:],
                        in_offset=bass.IndirectOffsetOnAxis(ap=ib_all[:, t : t + 1], axis=0),
                    )

                # PE-transpose (bf16, 1 cyc/row) to feature-major [768, 128]
                aT = midpool.tile([P, H], bf16, tag="aT")
                bT = midpool.tile([P, H], bf16, tag="bT")
                for src, dstT, tg in ((ga, aT, "tpa"), (gb, bT, "tpb")):
                    for half in range(2):
                        tp = tpsum.tile([P, 3 * P], bf16, tag=tg, space="PSUM")
                        for c in range(3):
                            cc = half * 3 + c
                            nc.tensor.transpose(
                                out=tp[:, c * P : (c + 1) * P],
                                in_=src[:, cc * P : (cc + 1) * P],
                                identity=identb[:],
                            )
                        nc.scalar.copy(
                            out=dstT[:, half * 3 * P : (half + 1) * 3 * P], in_=tp[:]
                        )

                dif = midpool.tile([P, H], bf16, tag="dif")
                prd = midpool.tile([P, H], bf16, tag="prd")
                nc.vector.tensor_sub(out=dif[:], in0=aT[:], in1=bT[:])
                nc.scalar.activation(out=dif[:], in_=dif[:], func=AF.Abs)
                nc.vector.tensor_mul(out=prd[:], in0=aT[:], in1=bT[:])

                # h = gelu(F @ W1 + b1), pairs-major [128, 768]
                h_sb = midpool.tile([P, H], f32, tag="h")
                for half in range(2):
                    ph = hpsum.tile([P, 384], f32, tag="ph", space="PSUM")
                    nsl = slice(half * 384, (half + 1) * 384)
                    for k in range(KC):
                        if k < 6:
                            lhs = aT[:, k * P : (k + 1) * P]
                        elif k < 12:
                            lhs = bT[:, (k - 6) * P : (k - 5) * P]
                        elif k < 18:
                            lhs = dif[:, (k - 12) * P : (k - 11) * P]
                        else:
                            lhs = prd[:, (k - 18) * P : (k - 17) * P]
                        nc.tensor.matmul(
                            out=ph[:],
                            lhsT=lhs,
                            rhs=w1_sb[:, k * H + half * 384 : k * H + half * 384 + 384],
                            start=(k == 0),
                            stop=(k == KC - 1),
                        )
                    nc.vector.tensor_add(
                        out=h_sb[:, nsl], in0=ph[:], in1=b1_rep[:, nsl]
                    )
                    nc.scalar.activation(
                        out=h_sb[:, nsl], in_=h_sb[:, nsl], func=AF.Gelu
                    )

                # logit[p] = sum_f h[p,f] * w2[f]  (fused mul + free-dim sum)
                trash = midpool.tile([P, H], f32, tag="trash")
                nc.vector.scalar_tensor_tensor(
                    out=trash[:],
                    in0=h_sb[:],
                    scalar=0.0,
                    in1=w2_rep[:],
                    op0=OP.add,
                    op1=OP.mult,
                    accum_out=logit_sb[:, t : t + 1],
                )

            # ---- epilogue: +b2, logits out ----
            nc.vector.tensor_scalar(
                out=logit_sb[:], in0=logit_sb[:], scalar1=b2_rep[:, :1],
                scalar2=None, op0=OP.add,
            )
            # logits out: transpose [128, T] -> [T, 128] then contiguous DMA
            ltp = tpsum.tile([P, P], f32, tag="tpa", space="PSUM")
            nc.tensor.transpose(out=ltp[:n_tiles, :], in_=logit_sb[:], identity=ident[:])
            lo_sb = wpool.tile([P, P], f32, tag="losb")
            nc.scalar.copy(out=lo_sb[:n_tiles, :], in_=ltp[:n_tiles, :])
            nc.sync.dma_start(
                out=logits_out.rearrange("(t l) -> t l", l=P), in_=lo_sb[:n_tiles, :]
            )

    _split_multi_waits(nc, mybir)
    return nc


def _split_multi_waits(nc, mybir):
    """This walrus build allows at most one sync-wait per instruction:
    hoist extra waits onto same-engine NOPs placed just before."""
    uid = 0
    for bb in nc.main_func.blocks:
        new_list = []
        for inst in bb.instructions:
            si = inst.sync_info
            if si is not None and si.on_wait and len(si.on_wait) > 1:
                waits = list(si.on_wait)
                for sw in waits[:-1]:
                    uid += 1
                    new_list.append(
                        mybir.InstNoOp(
                            name=f"I-wsplit-{uid}",
                            sync_info=mybir.SyncInfo(on_wait=[sw], on_update=[]),
                            bass_nofuse=True,
                            engine=inst.engine,
                        )
                    )
                si.on_wait = waits[-1:]
            new_list.append(inst)
        bb.instructions[:] = new_list


def kernel(node_embeddings, pair_a, pair_b, labels, W1, b1, W2, b2):
    from concourse.bass_utils import run_bass_kernel_spmd

    node_embeddings = np.ascontiguousarray(np.asarray(node_embeddings, dtype=np.float32))
    pair_a = np.asarray(pair_a).astype(np.int32)
    pair_b = np.asarray(pair_b).astype(np.int32)
    labels_in = np.asarray(labels)
    import ml_dtypes

    W1 = np.ascontiguousarray(
        np.asarray(W1, dtype=np.float32).astype(ml_dtypes.bfloat16)
    )
    b1v = np.asarray(b1, dtype=np.float32).reshape(-1)
    W2v = np.asarray(W2, dtype=np.float32).reshape(-1)
    b2v = np.asarray(b2, dtype=np.float32).reshape(-1)

    n_nodes = node_embeddings.shape[0]
    n_pairs = pair_a.shape[0]
    per_core = -(-n_pairs // N_CORES)
    n_tiles = -(-per_core // P)
    npc = n_tiles * P  # padded pairs per core

    key = (n_nodes, n_tiles)
    if key not in _CACHE:
        _CACHE[key] = _build_program(n_nodes, n_tiles)
    nc = _CACHE[key]

    lab_f = labels_in.astype(np.float32)
    in_maps = []
    for c in range(N_CORES):
        lo = c * per_core
        hi = min(lo + per_core, n_pairs)
        n = hi - lo
        pa_c = np.zeros(npc, dtype=np.int32)
        pb_c = np.zeros(npc, dtype=np.int32)
        pa_c[:n] = pair_a[lo:hi]
        pb_c[:n] = pair_b[lo:hi]
        pa_c = np.ascontiguousarray(pa_c.reshape(n_tiles, P).T)
        pb_c = np.ascontiguousarray(pb_c.reshape(n_tiles, P).T)
        in_maps.append(
            {
                "emb": node_embeddings,
                "pa": pa_c,
                "pb": pb_c,
                "w1": W1,
                "b1": b1v,
                "w2": W2v,
                "b2": b2v,
            }
        )

    import time as _time

    res = None
    for attempt, delay in ((0, 0.0), (1, 5.0), (2, 15.0)):
        try:
            if delay:
                _time.sleep(delay)
            res = run_bass_kernel_spmd(nc, in_maps, list(range(N_CORES)))
            break
        except Exception:
            # transient device wedge (e.g. NRT_EXEC_UNIT_UNRECOVERABLE)
            if attempt == 2:
                raise
    global LAST_RESULTS
    LAST_RESULTS = res

    logits = np.empty(n_pairs, dtype=np.float32)
    for c in range(N_CORES):
        lo = c * per_core
        hi = min(lo + per_core, n_pairs)
        logits[lo:hi] = res.results[c]["logits_out"][: hi - lo]

    # BCE-with-logits, mean reduction (stable form), as part of unsharding
    l64 = logits.astype(np.float64)
    y64 = lab_f.astype(np.float64)
    terms = np.maximum(l64, 0.0) - l64 * y64 + np.log1p(np.exp(-np.abs(l64)))
    loss = np.float32(terms.mean())
    return loss, logits
